# revision 49
# baseline (speedup 1.0000x reference)
"""Trainium2 Bass kernel for nn_CNN_Nested (W2NER-style CNN scorer).

Math (reference):
  head = leaky(wr @ head_w.T + head_b); tail likewise           [B,N,D]
  scores1[b,(h,d),l,k] = sum_{x,y} head[b,l,h,x] U[h,d,x,y] tail[b,k,h,y]
  scores2[b,c,m,n] = h_aug@Wh.T (bcast n) + t_aug@Wt.T (bcast m) + size-emb
  out = down_w @ (scores1+scores2) + down_b                     [B,OUT,N,N]

down_fc is linear => fold down_w into the constants on the host:
  U'[o,h,x,y] = sum_d down_w[o,h*HD+d] U[h,d,x,y]
  WhD = down_w @ Wh, WtD = down_w @ Wt               (tiny)
  E[o,m,n] = (size_emb @ (down_w@Ws).T)[clip(n-m)+15, o] + down_b[o]
Per (b, o-pair p = {o0,o1}), with out columns laid out [o0 n | o1 n]:
  ob[m, :] = headT_A^T @ gA  +  headT_B^T @ gB
where gA/gB = blockdiag(U')^T @ tailT (the biaffine term).  All broadcast
terms ride K=128-padded operands: tailT_A/B carry ones rows (via memset at
32-aligned bases) paired with WhD rows in bd, so gA/gB arrive with the
head-projection A'[o,k] pre-added; headT_B's ones row (partition 96) picks
up gB row 96 = B'-pair (tail projection + consts), produced by 4 M=65
matmuls whose outputs land at 32-aligned partitions and are engine-copied
(no DMA).  E is added during PSUM eviction by one fused vector add.

3 input DMAs (blob1, weights-pack, E-pack) and 3 output DMAs per core;
everything else stays on-chip.  All matmuls K=128 bf16 (PSUM f32).  A
warmup matmul stream opens the HAM clock gate.

Sharding: 8 cores = B(4) x o-half(2x6). No collectives. Full inputs in,
full output out. Hardcoded B=4,N=256,H=768,D=200,NH=5,HD=40,OUT=12.
"""

import os
import numpy as np

B, N, H = 4, 256, 768
D, NH, HD, SZ, OUT = 200, 5, 40, 25, 12
N_POS = 30
OH = OUT // 2          # o's per core
NCORES = 8
GA, GB = 3 * HD, 2 * HD  # 120 / 80: d-rows in partition group A / B
SEG = N + D              # blob1 per-chunk segment: [wrt_k | tw_k]
N_WARM = 6               # PE warmup matmuls (open the HAM clock gate)

# hwbd packed layout (columns)
HW_OFF = 0               # head_w pack        [128, 6*D]
SM_OFF = 6 * D           # activation biases  [128, 32]
PJ_OFF = SM_OFF + 32     # proj blocks LA|LB|RA|RB  [128, 4*65]
BDA_OFF = PJ_OFF + 260   # bd_a               [128, OH*GA]
BDB_OFF = BDA_OFF + OH * GA   # bd_b          [128, OH*GB]
HB_COLS = BDB_OFF + OH * GB

_cache = {}
LAST_RESULT = None


def _build_module(has_bias: bool):
    import concourse.bacc as bacc
    import concourse.mybir as mybir
    import concourse.tile as tile
    from concourse.bass import ts
    from contextlib import ExitStack

    dt = mybir.dt
    f32 = dt.float32
    bf = dt.bfloat16
    Act = mybir.ActivationFunctionType

    nc = bacc.Bacc("TRN2", target_bir_lowering=False, debug=False,
                   enable_asserts=False, enable_partition_id=False)

    b1_d = nc.dram_tensor("blob1", [128, 6 * SEG], bf, kind="ExternalInput").ap()
    hb_d = nc.dram_tensor("hwbd", [128, HB_COLS], bf, kind="ExternalInput").ap()
    e_d = nc.dram_tensor("e_pack", [128, OH * 512], bf,
                         kind="ExternalInput").ap()
    out_d = nc.dram_tensor("out", [3, 128, 1024], bf, kind="ExternalOutput").ap()

    with tile.TileContext(nc) as tc, ExitStack() as ctx:
        sb = ctx.enter_context(tc.tile_pool(name="sb", bufs=1))
        # Single PSUM pool for the whole kernel: 8 banks shared via tags
        # (ta/tb x2 + ha/hb/warm/pap x1).  No pool transitions — those
        # serialize the start of the next phase behind the previous
        # pool's last reader.
        pa = ctx.enter_context(tc.tile_pool(name="pa", bufs=1, space="PSUM"))

        # ---- tiles + constant-row memsets (32-aligned bases only) ---------
        scratch = sb.tile([128, 512], bf, tag="warm", name="warm")
        nc.vector.memset(scratch[:], 0.0)

        headT_A = sb.tile([GA, N], bf, tag="hTA", name="hTA")
        headT_B = sb.tile([97, N], bf, tag="hTB", name="hTB")
        tailT_A = sb.tile([GA + 1, N], bf, tag="tTA", name="tTA")
        tailT_B = sb.tile([97, N], bf, tag="tTB", name="tTB")
        nc.vector.memset(tailT_A[96:GA + 1, :], 1.0)  # row 120 stays ones
        nc.vector.memset(tailT_B[64:97, :], 1.0)      # rows 80-96 stay ones
        nc.vector.memset(headT_B[64:97, :], 0.0)
        nc.vector.memset(headT_B[96:97, :], 1.0)  # ones row at partition 96

        gAt = [sb.tile([GA, 512], bf, tag=f"gA{p}", name=f"gA{p}")
               for p in range(3)]
        gBt = [sb.tile([97, 512], bf, tag=f"gB{p}", name=f"gB{p}")
               for p in range(3)]
        for p in range(3):
            nc.vector.memset(gBt[p][64:97, :], 0.0)

        # ---- input DMA issues: one per tensor, ordered by first use.
        # blob1 gates everything; hwbd/e are STAGGERED (explicit deps on
        # warmup matmuls / tail MLP below) so their bytes don't steal HBM
        # bandwidth from blob1's transfer window.
        b1_s = sb.tile([128, 6 * SEG], bf, tag="b1", name="b1")
        nc.sync.dma_start(b1_s[:], b1_d[:, :])
        hb_s = sb.tile([128, HB_COLS], bf, tag="hb", name="hb")
        # split: head weights (needed right after the tail MLP) complete
        # ~1.5us before the proj/bd constants
        hb1_dma = nc.scalar.dma_start(hb_s[:, 0:SM_OFF], hb_d[:, 0:SM_OFF])
        hb2_dma = nc.scalar.dma_start(hb_s[:, SM_OFF:], hb_d[:, SM_OFF:])
        e_s = sb.tile([128, OH * 512], bf, tag="es", name="es")
        e_dma = nc.sync.dma_start(e_s[:], e_d[:, :])

        # ---- PE warmup: HAM clock gate opens after ~3.4us sustained -------
        warm_mms = []

        def dummy():
            wps = pa.tile([128, 512], f32, tag="wp", name="wps", bufs=1)
            mi = nc.tensor.matmul(wps[:], scratch[:, 0:128], scratch[:],
                                  start=True, stop=True)
            warm_mms.append(mi)

        for _ in range(N_WARM):
            dummy()

        def wrT(k):
            return b1_s[:, k * SEG:k * SEG + N]

        def tw_slice(k, off, sz):
            return b1_s[:, k * SEG + N + off:k * SEG + N + off + sz]

        def hw_slice(k, off, sz):
            c = HW_OFF + k * D + off
            return hb_s[:, c:c + sz]

        # ---- headT/tailT = leaky(w @ wr^T (+ b)), [d, l] layout -----------
        ps_mlp = {}
        mlp_jobs = [('ta', tw_slice, 'tb_a', 0, GA, tailT_A[0:GA, :]),
                    ('tb', tw_slice, 'tb_b', GA, GB, tailT_B[0:GB, :]),
                    ('ha', hw_slice, 'hb_a', 0, GA, headT_A[0:GA, :]),
                    ('hb', hw_slice, 'hb_b', GA, GB, headT_B[0:GB, :])]
        for key, wsl, bname, off, sz, dst in mlp_jobs:
            bufs = 2 if key in ('ta', 'tb') else 1
            ps_mlp[key] = pa.tile([sz, N], f32, tag=key,
                                  name=f"pm{key}", bufs=bufs)

        def mlp_mms(jobs):
            last = None
            for hk in range(6):
                for key, wsl, bname, off, sz, dst in jobs:
                    last = nc.tensor.matmul(ps_mlp[key][:], wsl(hk, off, sz),
                                            wrT(hk), start=(hk == 0),
                                            stop=(hk == 5))
            return last

        def mlp_act(jobs):
            for key, wsl, bname, off, sz, dst in jobs:
                if has_bias:
                    bc = {'hb_a': 24, 'hb_b': 25,
                          'tb_a': 26, 'tb_b': 27}[bname]
                    nc.scalar.activation(dst, ps_mlp[key][:], Act.Lrelu,
                                         bias=hb_s[0:sz, SM_OFF + bc:
                                                   SM_OFF + bc + 1],
                                         alpha=0.01)
                else:
                    nc.scalar.activation(dst, ps_mlp[key][:], Act.Lrelu,
                                         alpha=0.01)

        mlp_mms(mlp_jobs[:2])                        # tail first
        mlp_act(mlp_jobs[:2])

        # proj (right after tail acts): B''-pair rows at 32-aligned
        # partitions {0,32,64} of pap4; column halves [o_even n | o_odd n];
        # then aligned engine copies into gBt row 96 (pairs with headT_B's
        # ones row).
        pap4 = pa.tile([65, 512], f32, tag="pap", name="pap", bufs=1)
        nc.tensor.matmul(pap4[:, 0:N], hb_s[0:GA + 1, PJ_OFF:PJ_OFF + 65],
                         tailT_A[:], start=True, stop=False)
        nc.tensor.matmul(pap4[:, 0:N], hb_s[0:97, PJ_OFF + 65:PJ_OFF + 130],
                         tailT_B[:], start=False, stop=True)
        nc.tensor.matmul(pap4[:, N:2 * N],
                         hb_s[0:GA + 1, PJ_OFF + 130:PJ_OFF + 195],
                         tailT_A[:], start=True, stop=False)
        nc.tensor.matmul(pap4[:, N:2 * N],
                         hb_s[0:97, PJ_OFF + 195:PJ_OFF + 260],
                         tailT_B[:], start=False, stop=True)
        nc.vector.tensor_copy(gBt[0][96:97, :], pap4[0:1, :])

        mlp_mms(mlp_jobs[2:])                        # head
        mlp_act(mlp_jobs[2:])

        def g_build(p):
            gA, gB = gAt[p], gBt[p]
            psa = pa.tile([GA, 512], f32, tag="ta", name="psga", bufs=2)
            psb = pa.tile([GB, 512], f32, tag="tb", name="psgb", bufs=2)
            for half in range(2):
                j = 2 * p + half
                nc.tensor.matmul(psa[:, ts(half, N)],
                                 hb_s[0:GA + 1, BDA_OFF + j * GA:
                                      BDA_OFF + (j + 1) * GA],
                                 tailT_A[:], start=True, stop=True)
                nc.tensor.matmul(psb[:, ts(half, N)],
                                 hb_s[0:97, BDB_OFF + j * GB:
                                      BDB_OFF + (j + 1) * GB],
                                 tailT_B[:], start=True, stop=True)
            nc.vector.tensor_copy(gA[:, :], psa[:])
            nc.scalar.copy(gB[0:GB, :], psb[:])

        def out_bank(p):
            os_p = sb.tile([128, 1024], bf, tag=f"os{p}", name=f"os{p}")
            for lt in range(2):
                j = 2 * p + lt
                tag = ('ha', 'hb', 'wp')[j % 3]
                ob = pa.tile([128, 512], f32, tag=tag, name=f"ob{j}", bufs=1)
                nc.tensor.matmul(ob[:], headT_A[:, ts(lt, 128)], gAt[p][:],
                                 start=True, stop=False)
                nc.tensor.matmul(ob[:], headT_B[:, ts(lt, 128)],
                                 gBt[p][:], start=False, stop=True)
                nc.vector.tensor_add(os_p[:, ts(lt, 512)], ob[:],
                                     e_s[:, ts(j, 512)])
            # one DMA per bank: output receipts serialize per queue
            # (~1.5us apiece) and the LAST receipt gates the epilogue
            # drain -> 3 receipts (2 sync + 1 scalar) beat 6
            eng = nc.scalar if p == 1 else nc.sync
            eng.dma_start(out_d[p], os_p[:])

        g_build(0)
        g_build(1)
        nc.scalar.copy(gBt[1][96:97, :], pap4[32:33, :])
        g_build(2)
        nc.scalar.copy(gBt[2][96:97, :], pap4[64:65, :])
        out_bank(0)
        out_bank(1)
        out_bank(2)

    nc.compile()
    return nc


def _get_module(has_bias: bool):
    key = ("mod", has_bias)
    if key not in _cache:
        _cache[key] = _build_module(has_bias)
    return _cache[key]


def _host_pack(head_w, head_b, tail_w, tail_b, U_mh, size_emb, W, down_w,
               down_b):
    """Fold down_w into the constants; build per-o-half bf16 tensors."""
    from ml_dtypes import bfloat16
    f64 = np.float64
    d1 = D + 1
    Wh, Wt, Ws = W[:, :d1], W[:, d1:2 * d1], W[:, 2 * d1:]
    WhD = (down_w.astype(f64) @ Wh.astype(f64)).astype(np.float32)   # [OUT,D+1]
    WtD = (down_w.astype(f64) @ Wt.astype(f64)).astype(np.float32)
    WsD = (down_w.astype(f64) @ Ws.astype(f64)).astype(np.float32)   # [OUT,SZ]
    ct = (size_emb.astype(f64) @ WsD.T.astype(f64)).astype(np.float32)
    dw_r = down_w.reshape(OUT, NH, HD)
    Up = np.einsum('ohd,hdxy->ohxy', dw_r.astype(f64),
                   U_mh.astype(f64)).astype(np.float32)              # [OUT,NH,HD,HD]

    idx = np.arange(N)
    span = np.clip(idx[None, :] - idx[:, None], -N_POS // 2,
                   N_POS // 2 - 1) + N_POS // 2
    E = ct[span].transpose(2, 0, 1) + down_b[:, None, None]          # [OUT,N,N]

    has_bias = bool(np.any(head_b) or np.any(tail_b))

    def pack_w(wmat):  # [D,H] -> [128, 6*200]
        return np.ascontiguousarray(
            wmat.T.reshape(6, 128, D).transpose(1, 0, 2).reshape(128, 6 * D))

    hw_pack = pack_w(head_w)
    twp = pack_w(tail_w)
    blob1 = np.zeros((128, 6 * SEG), np.float32)
    for k in range(6):
        blob1[:, k * SEG + N:(k + 1) * SEG] = twp[:, k * D:(k + 1) * D]
    blob1 = blob1.astype(bfloat16)

    per_oh = []
    for oh in range(2):
        osl = slice(oh * OH, (oh + 1) * OH)
        UpS = Up[osl]                        # [6,5,40,40]
        WhDs = WhD[osl]                      # [6,201]
        WtDs = WtD[osl]

        bd = np.zeros((128, OH * GA + OH * GB), np.float32)
        for j in range(OH):
            for h in range(3):
                bd[h * HD:(h + 1) * HD, j * GA + h * HD:j * GA + (h + 1) * HD] \
                    = UpS[j, h].T
            bd[GA, j * GA:(j + 1) * GA] = WhDs[j, 0:GA]
            for h in range(2):
                bd[h * HD:(h + 1) * HD,
                   OH * GA + j * GB + h * HD:OH * GA + j * GB + (h + 1) * HD] \
                    = UpS[j, 3 + h].T
            bd[96, OH * GA + j * GB:OH * GA + (j + 1) * GB] \
                = WhDs[j, GA:D]

        pj = np.zeros((128, 4 * 65), np.float32)
        for p in range(3):
            for o2 in range(2):                       # 0=left cols, 1=right
                jj = 2 * p + o2
                a_off, b_off = 130 * o2, 130 * o2 + 65
                pj[0:GA, a_off + 32 * p] = WtDs[jj, 0:GA]
                pj[GA, a_off + 32 * p] = WtDs[jj, D] + WhDs[jj, D]
                pj[0:GB, b_off + 32 * p] = WtDs[jj, GA:D]

        sm = np.zeros((128, 32), np.float32)
        if has_bias:
            sm[0:GA, 24:25] = head_b[0:GA, None]
            sm[0:GB, 25:26] = head_b[GA:D, None]
            sm[0:GA, 26:27] = tail_b[0:GA, None]
            sm[0:GB, 27:28] = tail_b[GA:D, None]

        hwbd = np.zeros((128, HB_COLS), np.float32)
        hwbd[:, HW_OFF:HW_OFF + 6 * D] = hw_pack
        hwbd[:, SM_OFF:SM_OFF + 32] = sm
        hwbd[:, PJ_OFF:PJ_OFF + 260] = pj
        hwbd[:, BDA_OFF:BDA_OFF + OH * GA] = bd[:, 0:OH * GA]
        hwbd[:, BDB_OFF:BDB_OFF + OH * GB] = bd[:, OH * GA:]

        e_pack = np.zeros((128, OH * 512), np.float32)
        for p in range(OH // 2):
            for lt in range(2):
                o0 = oh * OH + 2 * p
                c0 = (2 * p + lt) * 512
                e_pack[:, c0:c0 + N] = E[o0, lt * 128:(lt + 1) * 128, :]
                e_pack[:, c0 + N:c0 + 512] = E[o0 + 1,
                                               lt * 128:(lt + 1) * 128, :]

        per_oh.append((hwbd.astype(bfloat16), e_pack.astype(bfloat16)))

    return blob1, per_oh, has_bias


def _ensure_axon():
    """If a host-side jax.config pinned the cpu platform (e.g. to run the
    reference), switch back to the axon/neuron backend for the device run."""
    import jax
    try:
        if any(getattr(d, 'platform', '') == 'axon' for d in jax.devices()):
            return
    except Exception:
        pass
    try:
        import jax.extend
        jax.config.update('jax_platforms', 'axon')
        jax.extend.backend.clear_backends()
    except Exception:
        pass


def _make_in_maps(word_reps, blob1, per_oh):
    from ml_dtypes import bfloat16
    wrt_b = []
    for b in range(B):
        wrt = word_reps[b].T.reshape(6, 128, N).transpose(1, 0, 2) \
            .reshape(128, 6 * N)
        wrt_b.append(wrt.astype(bfloat16))
    in_maps = []
    for core in range(NCORES):
        b, oh = core // 2, core % 2
        hwbd, ep = per_oh[oh]
        b1 = blob1.copy()
        for k in range(6):
            b1[:, k * SEG:k * SEG + N] = wrt_b[b][:, k * N:(k + 1) * N]
        in_maps.append(dict(blob1=b1, hwbd=hwbd, e_pack=ep))
    return in_maps


def kernel(word_reps, cls_embeding=None, pieces_index=None, loss_mask=None,
           head_w=None, head_b=None, tail_w=None, tail_b=None, U_mh=None,
           size_emb=None, W=None, down_w=None, down_b=None, **_unused):
    global LAST_RESULT
    from concourse import bass_utils

    word_reps = np.asarray(word_reps, np.float32)
    args = [np.asarray(a, np.float32) for a in
            (head_w, head_b, tail_w, tail_b, U_mh, size_emb, W, down_w,
             down_b)]
    blob1, per_oh, has_bias = _host_pack(*args)

    nc = _get_module(has_bias)

    in_maps = _make_in_maps(word_reps, blob1, per_oh)
    _ensure_axon()

    trace = bool(os.environ.get("KERNEL_TRACE"))
    if trace:
        try:
            from antenv.axon_hooks import get_axon_ntff_profile_hook  # noqa
        except ImportError:
            trace = False
    res = bass_utils.run_bass_kernel_spmd(nc, in_maps, list(range(NCORES)),
                                          trace=trace)
    LAST_RESULT = res

    out = np.empty((B, OUT, N, N), np.float32)
    for core in range(NCORES):
        b, oh = core // 2, core % 2
        r = np.asarray(res.results[core]["out"], dtype=np.float32)
        r = r.reshape(3, 128, 2, 2, 256)          # [p, q, lt, o2, n]
        for p in range(3):
            for lt in range(2):
                for o2 in range(2):
                    o = oh * OH + 2 * p + o2
                    out[b, o, lt * 128:(lt + 1) * 128, :] = r[p, :, lt, o2, :]
    return out


# revision 52
# speedup vs baseline: 1.0045x; 1.0045x over previous
"""Trainium2 Bass kernel for nn_CNN_Nested (W2NER-style CNN scorer).

Math (reference):
  head = leaky(wr @ head_w.T + head_b); tail likewise           [B,N,D]
  scores1[b,(h,d),l,k] = sum_{x,y} head[b,l,h,x] U[h,d,x,y] tail[b,k,h,y]
  scores2[b,c,m,n] = h_aug@Wh.T (bcast n) + t_aug@Wt.T (bcast m) + size-emb
  out = down_w @ (scores1+scores2) + down_b                     [B,OUT,N,N]

down_fc is linear => fold down_w into the constants on the host:
  U'[o,h,x,y] = sum_d down_w[o,h*HD+d] U[h,d,x,y]
  WhD = down_w @ Wh, WtD = down_w @ Wt               (tiny)
  E[o,m,n] = (size_emb @ (down_w@Ws).T)[clip(n-m)+15, o] + down_b[o]
Per (b, o-pair p = {o0,o1}), with out columns laid out [o0 n | o1 n]:
  ob[m, :] = headT_A^T @ gA  +  headT_B^T @ gB
where gA/gB = blockdiag(U')^T @ tailT (the biaffine term).  All broadcast
terms ride K=128-padded operands: tailT_A/B carry ones rows (via memset at
32-aligned bases) paired with WhD rows in bd, so gA/gB arrive with the
head-projection A'[o,k] pre-added; headT_B's ones row (partition 96) picks
up gB row 96 = B'-pair (tail projection + consts), produced by 4 M=65
matmuls whose outputs land at 32-aligned partitions and are engine-copied
(no DMA).  E is added during PSUM eviction by one fused vector add.

3 input DMAs (blob1, weights-pack, E-pack) and 3 output DMAs per core;
everything else stays on-chip.  All matmuls K=128 bf16 (PSUM f32).  A
warmup matmul stream opens the HAM clock gate.

Sharding: 8 cores = B(4) x o-half(2x6). No collectives. Full inputs in,
full output out. Hardcoded B=4,N=256,H=768,D=200,NH=5,HD=40,OUT=12.
"""

import os
import numpy as np

B, N, H = 4, 256, 768
D, NH, HD, SZ, OUT = 200, 5, 40, 25, 12
N_POS = 30
OH = OUT // 2          # o's per core
NCORES = 8
GA, GB = 3 * HD, 2 * HD  # 120 / 80: d-rows in partition group A / B
SEG = N + D              # blob1 per-chunk segment: [wrt_k | tw_k]
N_WARM = 6               # PE warmup matmuls (open the HAM clock gate)

# hwbd packed layout (columns)
HW_OFF = 0               # head_w pack        [128, 6*D]
SM_OFF = 6 * D           # activation biases  [128, 32]
PJ_OFF = SM_OFF + 32     # proj blocks LA|LB|RA|RB  [128, 4*65]
BDA_OFF = PJ_OFF + 260   # bd_a               [128, OH*GA]
BDB_OFF = BDA_OFF + OH * GA   # bd_b          [128, OH*GB]
HB_COLS = BDB_OFF + OH * GB

_cache = {}
LAST_RESULT = None


def _build_module(has_bias: bool):
    import concourse.bacc as bacc
    import concourse.mybir as mybir
    import concourse.tile as tile
    from concourse.bass import ts
    from contextlib import ExitStack

    dt = mybir.dt
    f32 = dt.float32
    bf = dt.bfloat16
    Act = mybir.ActivationFunctionType

    nc = bacc.Bacc("TRN2", target_bir_lowering=False, debug=False,
                   enable_asserts=False, enable_partition_id=False)

    b1_d = nc.dram_tensor("blob1", [128, 6 * SEG], bf, kind="ExternalInput").ap()
    hb_d = nc.dram_tensor("hwbd", [128, HB_COLS], bf, kind="ExternalInput").ap()
    e_d = nc.dram_tensor("e_pack", [128, OH * 512], bf,
                         kind="ExternalInput").ap()
    out_d = nc.dram_tensor("out", [OH, 128, 512], bf, kind="ExternalOutput").ap()

    with tile.TileContext(nc) as tc, ExitStack() as ctx:
        sb = ctx.enter_context(tc.tile_pool(name="sb", bufs=1))
        # Single PSUM pool for the whole kernel: 8 banks shared via tags
        # (ta/tb x2 + ha/hb/warm/pap x1).  No pool transitions — those
        # serialize the start of the next phase behind the previous
        # pool's last reader.
        pa = ctx.enter_context(tc.tile_pool(name="pa", bufs=1, space="PSUM"))

        # ---- tiles + constant-row memsets (32-aligned bases only) ---------
        scratch = sb.tile([128, 512], bf, tag="warm", name="warm")
        nc.vector.memset(scratch[:], 0.0)

        headT_A = sb.tile([GA, N], bf, tag="hTA", name="hTA")
        headT_B = sb.tile([97, N], bf, tag="hTB", name="hTB")
        tailT_A = sb.tile([GA + 1, N], bf, tag="tTA", name="tTA")
        tailT_B = sb.tile([97, N], bf, tag="tTB", name="tTB")
        nc.vector.memset(tailT_A[96:GA + 1, :], 1.0)  # row 120 stays ones
        nc.vector.memset(tailT_B[64:97, :], 1.0)      # rows 80-96 stay ones
        nc.vector.memset(headT_B[64:97, :], 0.0)
        nc.vector.memset(headT_B[96:97, :], 1.0)  # ones row at partition 96

        gAt = [sb.tile([GA, 512], bf, tag=f"gA{p}", name=f"gA{p}")
               for p in range(3)]
        gBt = [sb.tile([97, 512], bf, tag=f"gB{p}", name=f"gB{p}")
               for p in range(3)]
        for p in range(3):
            nc.vector.memset(gBt[p][64:97, :], 0.0)

        # ---- input DMA issues: one per tensor, ordered by first use.
        # blob1 gates everything; hwbd/e are STAGGERED (explicit deps on
        # warmup matmuls / tail MLP below) so their bytes don't steal HBM
        # bandwidth from blob1's transfer window.
        b1_s = sb.tile([128, 6 * SEG], bf, tag="b1", name="b1")
        nc.sync.dma_start(b1_s[:], b1_d[:, :])
        hb_s = sb.tile([128, HB_COLS], bf, tag="hb", name="hb")
        # split: head weights (needed right after the tail MLP) complete
        # ~1.5us before the proj/bd constants
        hb1_dma = nc.scalar.dma_start(hb_s[:, 0:SM_OFF], hb_d[:, 0:SM_OFF])
        hb2_dma = nc.scalar.dma_start(hb_s[:, SM_OFF:], hb_d[:, SM_OFF:])
        e_s = sb.tile([128, OH * 512], bf, tag="es", name="es")
        e_dma = nc.sync.dma_start(e_s[:], e_d[:, :])

        # ---- PE warmup: HAM clock gate opens after ~3.4us sustained -------
        warm_mms = []

        def dummy():
            wps = pa.tile([128, 512], f32, tag="wp", name="wps", bufs=1)
            mi = nc.tensor.matmul(wps[:], scratch[:, 0:128], scratch[:],
                                  start=True, stop=True)
            warm_mms.append(mi)

        for _ in range(N_WARM):
            dummy()

        def wrT(k):
            return b1_s[:, k * SEG:k * SEG + N]

        def tw_slice(k, off, sz):
            return b1_s[:, k * SEG + N + off:k * SEG + N + off + sz]

        def hw_slice(k, off, sz):
            c = HW_OFF + k * D + off
            return hb_s[:, c:c + sz]

        # ---- headT/tailT = leaky(w @ wr^T (+ b)), [d, l] layout -----------
        ps_mlp = {}
        mlp_jobs = [('ta', tw_slice, 'tb_a', 0, GA, tailT_A[0:GA, :]),
                    ('tb', tw_slice, 'tb_b', GA, GB, tailT_B[0:GB, :]),
                    ('ha', hw_slice, 'hb_a', 0, GA, headT_A[0:GA, :]),
                    ('hb', hw_slice, 'hb_b', GA, GB, headT_B[0:GB, :])]
        for key, wsl, bname, off, sz, dst in mlp_jobs:
            bufs = 2 if key in ('ta', 'tb') else 1
            ps_mlp[key] = pa.tile([sz, N], f32, tag=key,
                                  name=f"pm{key}", bufs=bufs)

        def mlp_mms(jobs):
            last = None
            for hk in range(6):
                for key, wsl, bname, off, sz, dst in jobs:
                    last = nc.tensor.matmul(ps_mlp[key][:], wsl(hk, off, sz),
                                            wrT(hk), start=(hk == 0),
                                            stop=(hk == 5))
            return last

        def mlp_act(jobs):
            for key, wsl, bname, off, sz, dst in jobs:
                if has_bias:
                    bc = {'hb_a': 24, 'hb_b': 25,
                          'tb_a': 26, 'tb_b': 27}[bname]
                    nc.scalar.activation(dst, ps_mlp[key][:], Act.Lrelu,
                                         bias=hb_s[0:sz, SM_OFF + bc:
                                                   SM_OFF + bc + 1],
                                         alpha=0.01)
                else:
                    nc.scalar.activation(dst, ps_mlp[key][:], Act.Lrelu,
                                         alpha=0.01)

        mlp_mms(mlp_jobs[:2])                        # tail first
        mlp_act(mlp_jobs[:2])

        # proj (right after tail acts): B''-pair rows at 32-aligned
        # partitions {0,32,64} of pap4; column halves [o_even n | o_odd n];
        # then aligned engine copies into gBt row 96 (pairs with headT_B's
        # ones row).
        pap4 = pa.tile([65, 512], f32, tag="pap", name="pap", bufs=1)
        nc.tensor.matmul(pap4[:, 0:N], hb_s[0:GA + 1, PJ_OFF:PJ_OFF + 65],
                         tailT_A[:], start=True, stop=False)
        nc.tensor.matmul(pap4[:, 0:N], hb_s[0:97, PJ_OFF + 65:PJ_OFF + 130],
                         tailT_B[:], start=False, stop=True)
        nc.tensor.matmul(pap4[:, N:2 * N],
                         hb_s[0:GA + 1, PJ_OFF + 130:PJ_OFF + 195],
                         tailT_A[:], start=True, stop=False)
        nc.tensor.matmul(pap4[:, N:2 * N],
                         hb_s[0:97, PJ_OFF + 195:PJ_OFF + 260],
                         tailT_B[:], start=False, stop=True)
        nc.vector.tensor_copy(gBt[0][96:97, :], pap4[0:1, :])

        mlp_mms(mlp_jobs[2:])                        # head
        mlp_act(mlp_jobs[2:])

        def g_build(p):
            gA, gB = gAt[p], gBt[p]
            psa = pa.tile([GA, 512], f32, tag="ta", name="psga", bufs=2)
            psb = pa.tile([GB, 512], f32, tag="tb", name="psgb", bufs=2)
            for half in range(2):
                j = 2 * p + half
                nc.tensor.matmul(psa[:, ts(half, N)],
                                 hb_s[0:GA + 1, BDA_OFF + j * GA:
                                      BDA_OFF + (j + 1) * GA],
                                 tailT_A[:], start=True, stop=True)
                nc.tensor.matmul(psb[:, ts(half, N)],
                                 hb_s[0:97, BDB_OFF + j * GB:
                                      BDB_OFF + (j + 1) * GB],
                                 tailT_B[:], start=True, stop=True)
            nc.vector.tensor_copy(gA[:, :], psa[:])
            nc.scalar.copy(gB[0:GB, :], psb[:])

        def out_bank(p):
            for lt in range(2):
                j = 2 * p + lt
                tag = ('ha', 'hb', 'wp')[j % 3]
                ob = pa.tile([128, 512], f32, tag=tag, name=f"ob{j}", bufs=1)
                nc.tensor.matmul(ob[:], headT_A[:, ts(lt, 128)], gAt[p][:],
                                 start=True, stop=False)
                nc.tensor.matmul(ob[:], headT_B[:, ts(lt, 128)],
                                 gBt[p][:], start=False, stop=True)
                osh = sb.tile([128, 512], bf, tag=f"os{j}", name=f"os{j}")
                nc.vector.tensor_add(osh[:], ob[:], e_s[:, ts(j, 512)])
                eng = nc.scalar if j % 2 else nc.sync
                eng.dma_start(out_d[j], osh[:])

        g_build(0)
        g_build(1)
        nc.scalar.copy(gBt[1][96:97, :], pap4[32:33, :])
        g_build(2)
        nc.scalar.copy(gBt[2][96:97, :], pap4[64:65, :])
        out_bank(0)
        out_bank(1)
        out_bank(2)

    nc.compile()
    return nc


def _get_module(has_bias: bool):
    key = ("mod", has_bias)
    if key not in _cache:
        _cache[key] = _build_module(has_bias)
    return _cache[key]


def _host_pack(head_w, head_b, tail_w, tail_b, U_mh, size_emb, W, down_w,
               down_b):
    """Fold down_w into the constants; build per-o-half bf16 tensors."""
    from ml_dtypes import bfloat16
    f64 = np.float64
    d1 = D + 1
    Wh, Wt, Ws = W[:, :d1], W[:, d1:2 * d1], W[:, 2 * d1:]
    WhD = (down_w.astype(f64) @ Wh.astype(f64)).astype(np.float32)   # [OUT,D+1]
    WtD = (down_w.astype(f64) @ Wt.astype(f64)).astype(np.float32)
    WsD = (down_w.astype(f64) @ Ws.astype(f64)).astype(np.float32)   # [OUT,SZ]
    ct = (size_emb.astype(f64) @ WsD.T.astype(f64)).astype(np.float32)
    dw_r = down_w.reshape(OUT, NH, HD)
    Up = np.einsum('ohd,hdxy->ohxy', dw_r.astype(f64),
                   U_mh.astype(f64)).astype(np.float32)              # [OUT,NH,HD,HD]

    idx = np.arange(N)
    span = np.clip(idx[None, :] - idx[:, None], -N_POS // 2,
                   N_POS // 2 - 1) + N_POS // 2
    E = ct[span].transpose(2, 0, 1) + down_b[:, None, None]          # [OUT,N,N]

    has_bias = bool(np.any(head_b) or np.any(tail_b))

    def pack_w(wmat):  # [D,H] -> [128, 6*200]
        return np.ascontiguousarray(
            wmat.T.reshape(6, 128, D).transpose(1, 0, 2).reshape(128, 6 * D))

    hw_pack = pack_w(head_w)
    twp = pack_w(tail_w)
    blob1 = np.zeros((128, 6 * SEG), np.float32)
    for k in range(6):
        blob1[:, k * SEG + N:(k + 1) * SEG] = twp[:, k * D:(k + 1) * D]
    blob1 = blob1.astype(bfloat16)

    per_oh = []
    for oh in range(2):
        osl = slice(oh * OH, (oh + 1) * OH)
        UpS = Up[osl]                        # [6,5,40,40]
        WhDs = WhD[osl]                      # [6,201]
        WtDs = WtD[osl]

        bd = np.zeros((128, OH * GA + OH * GB), np.float32)
        for j in range(OH):
            for h in range(3):
                bd[h * HD:(h + 1) * HD, j * GA + h * HD:j * GA + (h + 1) * HD] \
                    = UpS[j, h].T
            bd[GA, j * GA:(j + 1) * GA] = WhDs[j, 0:GA]
            for h in range(2):
                bd[h * HD:(h + 1) * HD,
                   OH * GA + j * GB + h * HD:OH * GA + j * GB + (h + 1) * HD] \
                    = UpS[j, 3 + h].T
            bd[96, OH * GA + j * GB:OH * GA + (j + 1) * GB] \
                = WhDs[j, GA:D]

        pj = np.zeros((128, 4 * 65), np.float32)
        for p in range(3):
            for o2 in range(2):                       # 0=left cols, 1=right
                jj = 2 * p + o2
                a_off, b_off = 130 * o2, 130 * o2 + 65
                pj[0:GA, a_off + 32 * p] = WtDs[jj, 0:GA]
                pj[GA, a_off + 32 * p] = WtDs[jj, D] + WhDs[jj, D]
                pj[0:GB, b_off + 32 * p] = WtDs[jj, GA:D]

        sm = np.zeros((128, 32), np.float32)
        if has_bias:
            sm[0:GA, 24:25] = head_b[0:GA, None]
            sm[0:GB, 25:26] = head_b[GA:D, None]
            sm[0:GA, 26:27] = tail_b[0:GA, None]
            sm[0:GB, 27:28] = tail_b[GA:D, None]

        hwbd = np.zeros((128, HB_COLS), np.float32)
        hwbd[:, HW_OFF:HW_OFF + 6 * D] = hw_pack
        hwbd[:, SM_OFF:SM_OFF + 32] = sm
        hwbd[:, PJ_OFF:PJ_OFF + 260] = pj
        hwbd[:, BDA_OFF:BDA_OFF + OH * GA] = bd[:, 0:OH * GA]
        hwbd[:, BDB_OFF:BDB_OFF + OH * GB] = bd[:, OH * GA:]

        e_pack = np.zeros((128, OH * 512), np.float32)
        for p in range(OH // 2):
            for lt in range(2):
                o0 = oh * OH + 2 * p
                c0 = (2 * p + lt) * 512
                e_pack[:, c0:c0 + N] = E[o0, lt * 128:(lt + 1) * 128, :]
                e_pack[:, c0 + N:c0 + 512] = E[o0 + 1,
                                               lt * 128:(lt + 1) * 128, :]

        per_oh.append((hwbd.astype(bfloat16), e_pack.astype(bfloat16)))

    return blob1, per_oh, has_bias


def _ensure_axon():
    """If a host-side jax.config pinned the cpu platform (e.g. to run the
    reference), switch back to the axon/neuron backend for the device run."""
    import jax
    try:
        if any(getattr(d, 'platform', '') == 'axon' for d in jax.devices()):
            return
    except Exception:
        pass
    try:
        import jax.extend
        jax.config.update('jax_platforms', 'axon')
        jax.extend.backend.clear_backends()
    except Exception:
        pass


def _make_in_maps(word_reps, blob1, per_oh):
    from ml_dtypes import bfloat16
    wrt_b = []
    for b in range(B):
        wrt = word_reps[b].T.reshape(6, 128, N).transpose(1, 0, 2) \
            .reshape(128, 6 * N)
        wrt_b.append(wrt.astype(bfloat16))
    in_maps = []
    for core in range(NCORES):
        b, oh = core // 2, core % 2
        hwbd, ep = per_oh[oh]
        b1 = blob1.copy()
        for k in range(6):
            b1[:, k * SEG:k * SEG + N] = wrt_b[b][:, k * N:(k + 1) * N]
        in_maps.append(dict(blob1=b1, hwbd=hwbd, e_pack=ep))
    return in_maps


def kernel(word_reps, cls_embeding=None, pieces_index=None, loss_mask=None,
           head_w=None, head_b=None, tail_w=None, tail_b=None, U_mh=None,
           size_emb=None, W=None, down_w=None, down_b=None, **_unused):
    global LAST_RESULT
    from concourse import bass_utils

    word_reps = np.asarray(word_reps, np.float32)
    args = [np.asarray(a, np.float32) for a in
            (head_w, head_b, tail_w, tail_b, U_mh, size_emb, W, down_w,
             down_b)]
    blob1, per_oh, has_bias = _host_pack(*args)

    nc = _get_module(has_bias)

    in_maps = _make_in_maps(word_reps, blob1, per_oh)
    _ensure_axon()

    trace = bool(os.environ.get("KERNEL_TRACE"))
    if trace:
        try:
            from antenv.axon_hooks import get_axon_ntff_profile_hook  # noqa
        except ImportError:
            trace = False
    res = bass_utils.run_bass_kernel_spmd(nc, in_maps, list(range(NCORES)),
                                          trace=trace)
    LAST_RESULT = res

    out = np.empty((B, OUT, N, N), np.float32)
    for core in range(NCORES):
        b, oh = core // 2, core % 2
        r = np.asarray(res.results[core]["out"], dtype=np.float32)
        r = r.reshape(3, 2, 128, 2, 256)          # [p, lt, q, o2, n]
        for p in range(3):
            for lt in range(2):
                for o2 in range(2):
                    o = oh * OH + 2 * p + o2
                    out[b, o, lt * 128:(lt + 1) * 128, :] = r[p, lt, :, o2, :]
    return out


# revision 54
# speedup vs baseline: 1.0127x; 1.0082x over previous
"""Trainium2 Bass kernel for nn_CNN_Nested (W2NER-style CNN scorer).

Math (reference):
  head = leaky(wr @ head_w.T + head_b); tail likewise           [B,N,D]
  scores1[b,(h,d),l,k] = sum_{x,y} head[b,l,h,x] U[h,d,x,y] tail[b,k,h,y]
  scores2[b,c,m,n] = h_aug@Wh.T (bcast n) + t_aug@Wt.T (bcast m) + size-emb
  out = down_w @ (scores1+scores2) + down_b                     [B,OUT,N,N]

down_fc is linear => fold down_w into the constants on the host:
  U'[o,h,x,y] = sum_d down_w[o,h*HD+d] U[h,d,x,y]
  WhD = down_w @ Wh, WtD = down_w @ Wt               (tiny)
  E[o,m,n] = (size_emb @ (down_w@Ws).T)[clip(n-m)+15, o] + down_b[o]
Per (b, o-pair p = {o0,o1}), with out columns laid out [o0 n | o1 n]:
  ob[m, :] = headT_A^T @ gA  +  headT_B^T @ gB
where gA/gB = blockdiag(U')^T @ tailT (the biaffine term).  All broadcast
terms ride K=128-padded operands: tailT_A/B carry ones rows (via memset at
32-aligned bases) paired with WhD rows in bd, so gA/gB arrive with the
head-projection A'[o,k] pre-added; headT_B's ones row (partition 96) picks
up gB row 96 = B'-pair (tail projection + consts), produced by 4 M=65
matmuls whose outputs land at 32-aligned partitions and are engine-copied
(no DMA).  E is added during PSUM eviction by one fused vector add.

3 input DMAs (blob1, weights-pack, E-pack) and 3 output DMAs per core;
everything else stays on-chip.  All matmuls K=128 bf16 (PSUM f32).  A
warmup matmul stream opens the HAM clock gate.

Sharding: 8 cores = B(4) x o-half(2x6). No collectives. Full inputs in,
full output out. Hardcoded B=4,N=256,H=768,D=200,NH=5,HD=40,OUT=12.
"""

import os
import numpy as np

B, N, H = 4, 256, 768
D, NH, HD, SZ, OUT = 200, 5, 40, 25, 12
N_POS = 30
OH = OUT // 2          # o's per core
NCORES = 8
GA, GB = 3 * HD, 2 * HD  # 120 / 80: d-rows in partition group A / B
SEG = N + D              # blob1 per-chunk segment: [wrt_k | tw_k]
N_WARM = 6               # PE warmup matmuls (open the HAM clock gate)

# hwbd packed layout (columns)
HW_OFF = 0               # head_w pack        [128, 6*D]
SM_OFF = 6 * D           # activation biases  [128, 32]
PJ_OFF = SM_OFF + 32     # proj blocks LA|LB|RA|RB  [128, 4*65]
BDA_OFF = PJ_OFF + 260   # bd_a               [128, OH*GA]
BDB_OFF = BDA_OFF + OH * GA   # bd_b          [128, OH*GB]
HB_COLS = BDB_OFF + OH * GB

_cache = {}
LAST_RESULT = None


def _build_module(has_bias: bool):
    import concourse.bacc as bacc
    import concourse.mybir as mybir
    import concourse.tile as tile
    from concourse.bass import ts
    from contextlib import ExitStack

    dt = mybir.dt
    f32 = dt.float32
    bf = dt.bfloat16
    Act = mybir.ActivationFunctionType

    nc = bacc.Bacc("TRN2", target_bir_lowering=False, debug=False,
                   enable_asserts=False, enable_partition_id=False)

    b1_d = nc.dram_tensor("blob1", [128, 6 * SEG], bf, kind="ExternalInput").ap()
    hb_d = nc.dram_tensor("hwbd", [128, HB_COLS], bf, kind="ExternalInput").ap()
    e_d = nc.dram_tensor("e_pack", [128, OH * 512], bf,
                         kind="ExternalInput").ap()
    out_d = nc.dram_tensor("out", [OH, 128, 512], bf, kind="ExternalOutput").ap()

    with tile.TileContext(nc) as tc, ExitStack() as ctx:
        sb = ctx.enter_context(tc.tile_pool(name="sb", bufs=1))
        # Single PSUM pool for the whole kernel: 8 banks shared via tags
        # (ta/tb x2 + ha/hb/warm/pap x1).  No pool transitions — those
        # serialize the start of the next phase behind the previous
        # pool's last reader.
        pa = ctx.enter_context(tc.tile_pool(name="pa", bufs=1, space="PSUM"))

        # ---- tiles + constant-row memsets (32-aligned bases only) ---------
        scratch = sb.tile([128, 512], bf, tag="warm", name="warm")
        nc.vector.memset(scratch[:], 0.0)

        headT_A = sb.tile([GA, N], bf, tag="hTA", name="hTA")
        headT_B = sb.tile([97, N], bf, tag="hTB", name="hTB")
        tailT_A = sb.tile([GA + 1, N], bf, tag="tTA", name="tTA")
        tailT_B = sb.tile([97, N], bf, tag="tTB", name="tTB")
        nc.vector.memset(tailT_A[96:GA + 1, :], 1.0)  # row 120 stays ones
        nc.vector.memset(tailT_B[64:97, :], 1.0)      # rows 80-96 stay ones
        nc.vector.memset(headT_B[64:97, :], 0.0)
        nc.vector.memset(headT_B[96:97, :], 1.0)  # ones row at partition 96

        gAt = [sb.tile([GA, 512], bf, tag=f"gA{p}", name=f"gA{p}")
               for p in range(3)]
        gBt = [sb.tile([97, 512], bf, tag=f"gB{p}", name=f"gB{p}")
               for p in range(3)]
        for p in range(3):
            nc.vector.memset(gBt[p][64:97, :], 0.0)

        # ---- input DMA issues: one per tensor, ordered by first use.
        # blob1 gates everything; hwbd/e are STAGGERED (explicit deps on
        # warmup matmuls / tail MLP below) so their bytes don't steal HBM
        # bandwidth from blob1's transfer window.
        b1_s = sb.tile([128, 6 * SEG], bf, tag="b1", name="b1")
        nc.sync.dma_start(b1_s[:], b1_d[:, :])
        hb_s = sb.tile([128, HB_COLS], bf, tag="hb", name="hb")
        # split: head weights (needed right after the tail MLP) complete
        # ~1.5us before the proj/bd constants
        hb1_dma = nc.scalar.dma_start(hb_s[:, 0:SM_OFF], hb_d[:, 0:SM_OFF])
        hb2_dma = nc.scalar.dma_start(hb_s[:, SM_OFF:], hb_d[:, SM_OFF:])
        e_s = sb.tile([128, OH * 512], bf, tag="es", name="es")
        e_dma = nc.sync.dma_start(e_s[:], e_d[:, :])

        # ---- PE warmup: HAM clock gate opens after ~3.4us sustained -------
        warm_mms = []

        def dummy():
            wps = pa.tile([128, 512], f32, tag="wp", name="wps", bufs=1)
            mi = nc.tensor.matmul(wps[:], scratch[:, 0:128], scratch[:],
                                  start=True, stop=True)
            warm_mms.append(mi)

        for _ in range(N_WARM):
            dummy()

        def wrT(k):
            return b1_s[:, k * SEG:k * SEG + N]

        def tw_slice(k, off, sz):
            return b1_s[:, k * SEG + N + off:k * SEG + N + off + sz]

        def hw_slice(k, off, sz):
            c = HW_OFF + k * D + off
            return hb_s[:, c:c + sz]

        # ---- headT/tailT = leaky(w @ wr^T (+ b)), [d, l] layout -----------
        ps_mlp = {}
        mlp_jobs = [('ta', tw_slice, 'tb_a', 0, GA, tailT_A[0:GA, :]),
                    ('tb', tw_slice, 'tb_b', GA, GB, tailT_B[0:GB, :]),
                    ('ha', hw_slice, 'hb_a', 0, GA, headT_A[0:GA, :]),
                    ('hb', hw_slice, 'hb_b', GA, GB, headT_B[0:GB, :])]
        for key, wsl, bname, off, sz, dst in mlp_jobs:
            bufs = 2 if key in ('ta', 'tb') else 1
            ps_mlp[key] = pa.tile([sz, N], f32, tag=key,
                                  name=f"pm{key}", bufs=bufs)

        def mlp_mms(jobs):
            last = None
            for hk in range(6):
                for key, wsl, bname, off, sz, dst in jobs:
                    last = nc.tensor.matmul(ps_mlp[key][:], wsl(hk, off, sz),
                                            wrT(hk), start=(hk == 0),
                                            stop=(hk == 5))
            return last

        def mlp_act(jobs):
            for key, wsl, bname, off, sz, dst in jobs:
                if has_bias:
                    bc = {'hb_a': 24, 'hb_b': 25,
                          'tb_a': 26, 'tb_b': 27}[bname]
                    nc.scalar.activation(dst, ps_mlp[key][:], Act.Lrelu,
                                         bias=hb_s[0:sz, SM_OFF + bc:
                                                   SM_OFF + bc + 1],
                                         alpha=0.01)
                else:
                    nc.scalar.activation(dst, ps_mlp[key][:], Act.Lrelu,
                                         alpha=0.01)

        mlp_mms(mlp_jobs[:2])                        # tail first
        mlp_act(mlp_jobs[:2])

        # proj (right after tail acts): B''-pair rows at 32-aligned
        # partitions {0,32,64} of pap4; column halves [o_even n | o_odd n];
        # then aligned engine copies into gBt row 96 (pairs with headT_B's
        # ones row).
        pap4 = pa.tile([65, 512], f32, tag="pap", name="pap", bufs=1)
        nc.tensor.matmul(pap4[:, 0:N], hb_s[0:GA + 1, PJ_OFF:PJ_OFF + 65],
                         tailT_A[:], start=True, stop=False)
        nc.tensor.matmul(pap4[:, 0:N], hb_s[0:97, PJ_OFF + 65:PJ_OFF + 130],
                         tailT_B[:], start=False, stop=True)
        nc.tensor.matmul(pap4[:, N:2 * N],
                         hb_s[0:GA + 1, PJ_OFF + 130:PJ_OFF + 195],
                         tailT_A[:], start=True, stop=False)
        nc.tensor.matmul(pap4[:, N:2 * N],
                         hb_s[0:97, PJ_OFF + 195:PJ_OFF + 260],
                         tailT_B[:], start=False, stop=True)
        nc.vector.tensor_copy(gBt[0][96:97, :], pap4[0:1, :])

        mlp_mms(mlp_jobs[2:])                        # head
        mlp_act(mlp_jobs[2:])

        def g_build(p):
            gA, gB = gAt[p], gBt[p]
            psa = pa.tile([GA, 512], f32, tag="ta", name="psga", bufs=2)
            psb = pa.tile([GB, 512], f32, tag="tb", name="psgb", bufs=2)
            for half in range(2):
                j = 2 * p + half
                nc.tensor.matmul(psa[:, ts(half, N)],
                                 hb_s[0:GA + 1, BDA_OFF + j * GA:
                                      BDA_OFF + (j + 1) * GA],
                                 tailT_A[:], start=True, stop=True)
                nc.tensor.matmul(psb[:, ts(half, N)],
                                 hb_s[0:97, BDB_OFF + j * GB:
                                      BDB_OFF + (j + 1) * GB],
                                 tailT_B[:], start=True, stop=True)
            nc.vector.tensor_copy(gA[:, :], psa[:])
            nc.scalar.copy(gB[0:GB, :], psb[:])

        def out_bank(p):
            for lt in range(2):
                j = 2 * p + lt
                tag = ('ha', 'hb', 'wp')[j % 3]
                ob = pa.tile([128, 512], f32, tag=tag, name=f"ob{j}", bufs=1)
                nc.tensor.matmul(ob[:], headT_A[:, ts(lt, 128)], gAt[p][:],
                                 start=True, stop=False)
                nc.tensor.matmul(ob[:], headT_B[:, ts(lt, 128)],
                                 gBt[p][:], start=False, stop=True)
                osh = sb.tile([128, 512], bf, tag=f"os{j}", name=f"os{j}")
                nc.vector.tensor_add(osh[:], ob[:], e_s[:, ts(j, 512)])
                eng = nc.scalar if j % 2 else nc.sync
                eng.dma_start(out_d[j], osh[:])

        g_build(0)
        g_build(1)
        nc.scalar.copy(gBt[1][96:97, :], pap4[32:33, :])
        g_build(2)
        nc.scalar.copy(gBt[2][96:97, :], pap4[64:65, :])
        out_bank(0)
        out_bank(1)
        out_bank(2)

    nc.compile()
    return nc


def _get_module(has_bias: bool):
    key = ("mod", has_bias)
    if key not in _cache:
        _cache[key] = _build_module(has_bias)
    return _cache[key]


def _host_pack(head_w, head_b, tail_w, tail_b, U_mh, size_emb, W, down_w,
               down_b):
    """Fold down_w into the constants; build per-o-half bf16 tensors."""
    from ml_dtypes import bfloat16
    f64 = np.float64
    d1 = D + 1
    Wh, Wt, Ws = W[:, :d1], W[:, d1:2 * d1], W[:, 2 * d1:]
    WhD = (down_w.astype(f64) @ Wh.astype(f64)).astype(np.float32)   # [OUT,D+1]
    WtD = (down_w.astype(f64) @ Wt.astype(f64)).astype(np.float32)
    WsD = (down_w.astype(f64) @ Ws.astype(f64)).astype(np.float32)   # [OUT,SZ]
    ct = (size_emb.astype(f64) @ WsD.T.astype(f64)).astype(np.float32)
    dw_r = down_w.reshape(OUT, NH, HD)
    Up = np.einsum('ohd,hdxy->ohxy', dw_r.astype(f64),
                   U_mh.astype(f64)).astype(np.float32)              # [OUT,NH,HD,HD]

    idx = np.arange(N)
    span = np.clip(idx[None, :] - idx[:, None], -N_POS // 2,
                   N_POS // 2 - 1) + N_POS // 2
    E = ct[span].transpose(2, 0, 1) + down_b[:, None, None]          # [OUT,N,N]

    has_bias = bool(np.any(head_b) or np.any(tail_b))

    def pack_w(wmat):  # [D,H] -> [128, 6*200]
        return np.ascontiguousarray(
            wmat.T.reshape(6, 128, D).transpose(1, 0, 2).reshape(128, 6 * D))

    hw_pack = pack_w(head_w)
    twp = pack_w(tail_w)
    blob1 = np.zeros((128, 6 * SEG), np.float32)
    for k in range(6):
        blob1[:, k * SEG + N:(k + 1) * SEG] = twp[:, k * D:(k + 1) * D]
    blob1 = blob1.astype(bfloat16)

    per_oh = []
    for oh in range(2):
        osl = slice(oh * OH, (oh + 1) * OH)
        UpS = Up[osl]                        # [6,5,40,40]
        WhDs = WhD[osl]                      # [6,201]
        WtDs = WtD[osl]

        bd = np.zeros((128, OH * GA + OH * GB), np.float32)
        for j in range(OH):
            for h in range(3):
                bd[h * HD:(h + 1) * HD, j * GA + h * HD:j * GA + (h + 1) * HD] \
                    = UpS[j, h].T
            bd[GA, j * GA:(j + 1) * GA] = WhDs[j, 0:GA]
            for h in range(2):
                bd[h * HD:(h + 1) * HD,
                   OH * GA + j * GB + h * HD:OH * GA + j * GB + (h + 1) * HD] \
                    = UpS[j, 3 + h].T
            bd[96, OH * GA + j * GB:OH * GA + (j + 1) * GB] \
                = WhDs[j, GA:D]

        pj = np.zeros((128, 4 * 65), np.float32)
        for p in range(3):
            for o2 in range(2):                       # 0=left cols, 1=right
                jj = 2 * p + o2
                a_off, b_off = 130 * o2, 130 * o2 + 65
                pj[0:GA, a_off + 32 * p] = WtDs[jj, 0:GA]
                pj[GA, a_off + 32 * p] = WtDs[jj, D] + WhDs[jj, D]
                pj[0:GB, b_off + 32 * p] = WtDs[jj, GA:D]

        sm = np.zeros((128, 32), np.float32)
        if has_bias:
            sm[0:GA, 24:25] = head_b[0:GA, None]
            sm[0:GB, 25:26] = head_b[GA:D, None]
            sm[0:GA, 26:27] = tail_b[0:GA, None]
            sm[0:GB, 27:28] = tail_b[GA:D, None]

        hwbd = np.zeros((128, HB_COLS), np.float32)
        hwbd[:, HW_OFF:HW_OFF + 6 * D] = hw_pack
        hwbd[:, SM_OFF:SM_OFF + 32] = sm
        hwbd[:, PJ_OFF:PJ_OFF + 260] = pj
        hwbd[:, BDA_OFF:BDA_OFF + OH * GA] = bd[:, 0:OH * GA]
        hwbd[:, BDB_OFF:BDB_OFF + OH * GB] = bd[:, OH * GA:]

        e_pack = np.zeros((128, OH * 512), np.float32)
        for p in range(OH // 2):
            for lt in range(2):
                o0 = oh * OH + 2 * p
                c0 = (2 * p + lt) * 512
                e_pack[:, c0:c0 + N] = E[o0, lt * 128:(lt + 1) * 128, :]
                e_pack[:, c0 + N:c0 + 512] = E[o0 + 1,
                                               lt * 128:(lt + 1) * 128, :]

        per_oh.append((hwbd.astype(bfloat16), e_pack.astype(bfloat16)))

    return blob1, per_oh, has_bias


def _ensure_axon():
    """If a host-side jax.config pinned the cpu platform (e.g. to run the
    reference), switch back to the axon/neuron backend for the device run."""
    import jax
    try:
        if any(getattr(d, 'platform', '') == 'axon' for d in jax.devices()):
            return
    except Exception:
        pass
    try:
        import jax.extend
        jax.config.update('jax_platforms', 'axon')
        jax.extend.backend.clear_backends()
    except Exception:
        pass


def _make_in_maps(word_reps, blob1, per_oh):
    from ml_dtypes import bfloat16
    wrt_b = []
    for b in range(B):
        wrt = word_reps[b].T.reshape(6, 128, N).transpose(1, 0, 2) \
            .reshape(128, 6 * N)
        wrt_b.append(wrt.astype(bfloat16))
    in_maps = []
    for core in range(NCORES):
        b, oh = core // 2, core % 2
        hwbd, ep = per_oh[oh]
        b1 = blob1.copy()
        for k in range(6):
            b1[:, k * SEG:k * SEG + N] = wrt_b[b][:, k * N:(k + 1) * N]
        in_maps.append(dict(blob1=b1, hwbd=hwbd, e_pack=ep))
    return in_maps


def kernel(word_reps, cls_embeding=None, pieces_index=None, loss_mask=None,
           head_w=None, head_b=None, tail_w=None, tail_b=None, U_mh=None,
           size_emb=None, W=None, down_w=None, down_b=None, **_unused):
    global LAST_RESULT
    from concourse import bass_utils

    word_reps = np.asarray(word_reps, np.float32)
    args = [np.asarray(a, np.float32) for a in
            (head_w, head_b, tail_w, tail_b, U_mh, size_emb, W, down_w,
             down_b)]
    blob1, per_oh, has_bias = _host_pack(*args)

    nc = _get_module(has_bias)

    in_maps = _make_in_maps(word_reps, blob1, per_oh)
    _ensure_axon()

    trace = bool(os.environ.get("KERNEL_TRACE"))
    if trace:
        try:
            from antenv.axon_hooks import get_axon_ntff_profile_hook  # noqa
        except ImportError:
            trace = False
    res = bass_utils.run_bass_kernel_spmd(nc, in_maps, list(range(NCORES)),
                                          trace=trace)
    LAST_RESULT = res

    out = np.empty((B, OUT, N, N), np.float32)
    for core in range(NCORES):
        b, oh = core // 2, core % 2
        r = np.asarray(res.results[core]["out"], dtype=np.float32)
        r = r.reshape(3, 2, 128, 2, 256)          # [p, lt, q, o2, n]
        for p in range(3):
            for lt in range(2):
                for o2 in range(2):
                    o = oh * OH + 2 * p + o2
                    out[b, o, lt * 128:(lt + 1) * 128, :] = r[p, lt, :, o2, :]
    return out
